# revision 27
# baseline (speedup 1.0000x reference)
"""Trainium2 Bass kernel for a transformer decoder layer — 8-way, zero-collective.

Sharding: pure data-parallel over tokens.  Core r owns rows
[512r, 512(r+1)) of the flattened [B*L, D] = [4096, 1024] token axis
(batch 0 = cores 0-3, batch 1 = cores 4-7).  Weights are fully replicated.

Design notes (vs the tensor-parallel baseline this replaces):
  - In the harness cost model a collective costs 15us + out_bytes/40GBps;
    the TP baseline spent ~1ms of its 1.47ms in AllGather/ReduceScatter.
    Every tensor a core needs besides its own activations is a kernel
    *input* already in HBM, so each core instead recomputes its batch's
    K projections locally (~55us of redundant PE work) and runs the whole
    layer with ZERO collectives.
  - Attention value runs in natural layout: ctx[t, hd] accumulates with
    lhsT = p (exp scores, [m, t]) and rhs = K-natural tiles [m, 64+1];
    the 65th column holds 1 for real context positions and 0 for padded
    ones, so it accumulates the softmax denominator Z AND implements the
    cross-attention mask exactly (pad contributes exp(0)*0 = 0 to both
    numerator and denominator).  No exp bias needed -> exp fuses over two
    m-tiles, halving the ACT access overhead in the softmax stream.
  - The attention cores are ACT(exp)-bound; the PE idle gap under them is
    filled by streaming the cross-attention K2^T projection + K-natural
    build (self core) and the ctx^T transposes (both cores) as filler.
  - PSUM accumulation: start=True marks the whole 2KB zero region, so
    every bank gets exactly one start (first write) and one stop (last).

Numerics: bf16 operands on the PE, fp32 PSUM accumulation, fp32
residual + LayerNorm (bf16 final store).  Host pre-transposes x^T/enc^T,
permutes attention weights head-major, pre-compacts the enc context.
"""

import sys

sys.path.insert(0, "/opt/trn_rl_repo")

import numpy as np

import concourse.bass as bass
import concourse.bacc as bacc
import concourse.mybir as mybir
import concourse.tile as tile
from concourse.bass_utils import run_bass_kernel_spmd
from concourse.masks import make_identity

dt = mybir.dt
AF = mybir.ActivationFunctionType
ALU = mybir.AluOpType

P = 128
D = 1024          # d_model
DT = D // P       # 8 input-channel tiles
H = 16            # heads
HD = 64           # head dim
CHT = D // P      # 8 channel tiles (2 heads each)
MLP = 4096
HTT = MLP // P    # 32 hidden tiles
B, L, M = 2, 2048, 2048
T = B * L
NCORES = 8
TS = T // NCORES  # 512 tokens per core
TT = TS // P      # 4 own-token tiles
NK = 512          # matmul free-dim chunk
MT = L // P       # 16 self-attention m-tiles
EPS = 1e-5

_PROGRAM_CACHE = {}


def _build_program(trivial_affine, trivial_ffb, NT):
    """NT: cross-attention context m-tiles (shared across batches; padded
    slots contribute exactly 0 via zeroed K-natural ones-columns)."""
    NC = NT * P           # cross context tokens (padded)
    NKC = (NC + NK - 1) // NK
    nc = bacc.Bacc(None)
    f32 = dt.float32
    rdt = dt.float32r
    b16 = dt.bfloat16

    def din(name, shape, d):
        return nc.declare_dram_parameter(name, list(shape), d, isOutput=False)

    xqT_d = din("xqT", [P, DT, NK], b16)    # own x^T (pmajor)
    xbT_d = din("xbT", [P, DT, L], b16)     # full-batch x^T (pmajor)
    xs_d = din("xs", [TS, D], f32)          # own x rows (residual)
    encT_d = din("encT", [P, DT, NC], b16)  # compacted enc^T (pmajor)
    mo_d = din("mo", [P, NT, 1], b16)       # 1 kept / 0 padded, per m token
    q1W_d = din("q1W", [P, DT, D], b16)
    w1W_d = din("w1W", [P, DT, D], b16)
    o1W_d = din("o1W", [P, DT, D], b16)
    q2W_d = din("q2W", [P, DT, D], b16)
    w2W_d = din("w2W", [P, DT, D], b16)
    o2W_d = din("o2W", [P, DT, D], b16)
    ffW1_d = din("ffW1", [P, DT, MLP], b16)
    ffW2_d = din("ffW2", [P, HTT, D], b16)
    gb_d = {}
    if not trivial_affine:
        for nm in ("g1", "b1", "g2", "b2", "g3", "b3"):
            gb_d[nm] = din(nm + "b", [P, D], f32)
    if not trivial_ffb:
        ffb2b_d = din("ffb2b", [P, D], f32)
        ffb1h_d = din("ffb1h", [P, HTT], f32)
    out_d = nc.declare_dram_parameter("out", [TS, D], b16, isOutput=True)

    lp = nc.allow_low_precision(reason="bf16 weights/activations")
    lp.__enter__()
    with tile.TileContext(nc) as tc:
        cpool = tc.alloc_tile_pool(name="const", bufs=1)
        small = tc.alloc_tile_pool(name="small", bufs=3)

        ident_f = cpool.tile([P, P], f32)
        make_identity(nc, ident_f[:])
        ident_b = cpool.tile([P, P], b16)
        nc.vector.tensor_copy(ident_b[:], ident_f[:])
        ident_r = cpool.tile([P, P], rdt)
        nc.vector.tensor_copy(ident_r[:], ident_f[:])
        mo_t = cpool.tile([P, NT, 1], b16)
        if not trivial_ffb:
            ffb1h_t = cpool.tile([P, HTT], f32)
            nc.sync.dma_start(ffb1h_t[:], ffb1h_d[:])

        # ---------------- LayerNorm helpers ----------------
        def ln_stats(rsb_tt, st, tt):
            for h in range(2):
                nc.vector.bn_stats(st[:, tt, h, :],
                                   rsb_tt[:, h * NK:(h + 1) * NK])

        def ln_finish(rsb, st, x_out, gkey, post_tt=None):
            mv = small.tile([P, TT, 2], f32, tag="ln_mv")
            for tt in range(TT):
                nc.vector.bn_aggr(mv[:, tt, :], st[:, tt, :, :])
            t = small.tile([P, TT], f32, tag="ln_t")
            nc.vector.tensor_scalar_add(t[:], mv[:, :, 1], EPS)
            s = small.tile([P, TT], f32, tag="ln_s")
            nc.scalar.sqrt(s[:], t[:])
            r0 = small.tile([P, TT], f32, tag="ln_r0")
            nc.vector.reciprocal(r0[:], s[:])
            # one Newton step: r1 = r0 * (1.5 - 0.5 * t * r0^2)
            u = small.tile([P, TT], f32, tag="ln_u")
            nc.vector.tensor_tensor(out=u[:], in0=t[:], in1=r0[:], op=ALU.mult)
            nc.vector.tensor_tensor(out=u[:], in0=u[:], in1=r0[:], op=ALU.mult)
            nc.vector.tensor_scalar(u[:], u[:], -0.5, 1.5, ALU.mult, ALU.add)
            r1 = small.tile([P, TT], f32, tag="ln_r1")
            nc.vector.tensor_tensor(out=r1[:], in0=r0[:], in1=u[:], op=ALU.mult)
            for tt in range(TT):
                if trivial_affine:
                    nc.vector.tensor_scalar(
                        x_out[:, tt, :], rsb[:, tt, :], mv[:, tt, 0:1],
                        r1[:, tt:tt + 1], ALU.subtract, ALU.mult)
                else:
                    g_t = small.tile([P, D], f32, tag="ln_g", bufs=2)
                    nc.sync.dma_start(g_t[:], gb_d["g" + gkey][:])
                    b_t = small.tile([P, D], f32, tag="ln_b", bufs=2)
                    nc.sync.dma_start(b_t[:], gb_d["b" + gkey][:])
                    nc.vector.tensor_scalar(
                        rsb[:, tt, :], rsb[:, tt, :], mv[:, tt, 0:1],
                        r1[:, tt:tt + 1], ALU.subtract, ALU.mult)
                    nc.vector.tensor_tensor(out=rsb[:, tt, :], in0=rsb[:, tt, :],
                                            in1=g_t[:], op=ALU.mult)
                    nc.vector.tensor_tensor(out=x_out[:, tt, :], in0=rsb[:, tt, :],
                                            in1=b_t[:], op=ALU.add)
                if post_tt is not None:
                    post_tt(tt)

        # ---------------- attention building blocks ----------------
        def proj_T(wt, rhs_t, dst, pp, nm, width, alt=0):
            """dst[:, cht, 0:width] = (W^T x^T) bf16 for all channel tiles."""
            for cht in range(CHT):
                ps = pp.tile([P, NK], f32, tag=f"ps_{nm}", bufs=3)
                for dti in range(DT):
                    nc.tensor.matmul(ps[:, 0:width],
                                     wt[:, dti, cht * P:(cht + 1) * P],
                                     rhs_t[:, dti, 0:width],
                                     start=(dti == 0), stop=(dti == DT - 1))
                if (cht + alt) % 2 == 0:
                    nc.vector.tensor_copy(dst[:, cht, 0:width], ps[:, 0:width])
                else:
                    nc.scalar.copy(dst[:, cht, 0:width], ps[:, 0:width])

        def knat_copy(kn_t, cht, g0, rem, src):
            """src: [P, rem*P] AP of transposed K tiles (psum)."""
            nc.vector.tensor_copy(
                kn_t[cht][:, g0:g0 + rem, :, 0:HD],
                src.rearrange("p (mt hd) -> p mt hd", hd=P)
                .rearrange("p mt (h c) -> p mt h c", c=HD))

        def attn_core(KT_t, kn_t, QT_t, ctxT, n_mt, pa, pat,
                      filler=None, gap_ns=600.0):
            """Scores + fused-exp softmax + natural-layout value for own 512
            queries, all 16 heads; normalized context transposed into
            ctxT [P, CHT, TS] per head-pair.  filler: generator emitting one
            PE work unit per next(), yielding its ns cost — pulled into the
            per-pair ACT-bound idle gap."""
            credit = 0.0
            npair = (n_mt + 1) // 2
            for hp in range(CHT):
                ctxp = [pa.tile([P, TT, P], f32, tag=f"ctx{j}", bufs=1,
                                name=f"ctx{j}") for j in range(2)]
                for mp in range(npair):
                    nmi = min(2, n_mt - 2 * mp)
                    s4 = pa.tile([P, 2, 2, NK], f32, tag="s4", bufs=1)
                    for mi in range(nmi):
                        mt = 2 * mp + mi
                        for j in range(2):
                            nc.tensor.matmul(
                                s4[:, mi, j, :],
                                KT_t[j * HD:(j + 1) * HD, hp,
                                     mt * P:(mt + 1) * P],
                                QT_t[j * HD:(j + 1) * HD, hp, :],
                                start=True, stop=True)
                    p4 = pat.tile([P, 2, 2, NK], b16, tag="p2", bufs=2,
                                  name="p2")
                    if nmi == 2:
                        nc.scalar.activation(p4[:], s4[:], AF.Exp, scale=0.125)
                    else:
                        nc.scalar.activation(p4[:, 0], s4[:, 0], AF.Exp,
                                             scale=0.125)
                    for mi in range(nmi):
                        mt = 2 * mp + mi
                        for j in range(2):
                            for ts4 in range(TT):
                                # one start/stop per 2KB PSUM zero region
                                nc.tensor.matmul(
                                    ctxp[j][:, ts4, 0:HD + 1],
                                    p4[:, mi, j, ts4 * P:(ts4 + 1) * P],
                                    kn_t[hp][:, mt, j, :],
                                    start=(mp == 0 and mi == 0 and ts4 == 0),
                                    stop=(mt == n_mt - 1 and ts4 == TT - 1))
                    if filler is not None:
                        credit += gap_ns * (nmi / 2.0)
                        while credit > 0:
                            c = next(filler, None)
                            if c is None:
                                filler = None
                                break
                            credit -= c
                rec = small.tile([P, 2, TT, 1], f32, tag="rec")
                for j in range(2):
                    nc.vector.reciprocal(rec[:, j], ctxp[j][:, :, HD:HD + 1])
                ctxh = pat.tile([P, TT, P], b16, tag="ctxh", bufs=2,
                                name="ctxh")
                for j in range(2):
                    for ts4 in range(TT):
                        nc.vector.tensor_scalar(
                            ctxh[:, ts4, j * HD:(j + 1) * HD],
                            ctxp[j][:, ts4, 0:HD],
                            rec[:, j, ts4], None, ALU.mult)
                tp = pa.tile([P, NK], b16, tag="tpb16", bufs=1)
                for ts4 in range(TT):
                    nc.tensor.transpose(tp[:, ts4 * P:(ts4 + 1) * P],
                                        ctxh[:, ts4, :], ident_b[:])
                nc.vector.tensor_copy(ctxT[:, hp, :], tp[:])
            if filler is not None:
                while next(filler, None) is not None:
                    pass

        def o_proj_ln(ctxT, ow_t, resid, x_out, gkey, pp, rsb, extra=None,
                      post_tt=None):
            st = small.tile([P, TT, 2, 6], f32, tag="ln_st")
            for tt in range(TT):
                po = pp.tile([P, D], f32, tag="po", bufs=2)
                for cht in range(CHT):
                    for dc in range(2):
                        nc.tensor.matmul(
                            po[:, dc * NK:(dc + 1) * NK],
                            ctxT[:, cht, tt * P:(tt + 1) * P],
                            ow_t[:, cht, dc * NK:(dc + 1) * NK],
                            start=(cht == 0), stop=(cht == CHT - 1))
                nc.vector.tensor_tensor(out=rsb[:, tt, :], in0=po[:],
                                        in1=resid[:, tt, :], op=ALU.add)
                if extra is not None:
                    nc.vector.tensor_tensor(out=rsb[:, tt, :], in0=rsb[:, tt, :],
                                            in1=extra[:], op=ALU.add)
                ln_stats(rsb[:, tt, :], st, tt)
            ln_finish(rsb, st, x_out, gkey, post_tt=post_tt)

        def transpose_own(x_t, xT, pp):
            """x_t [P, TT, D] (f32r) -> xT [P, CHT, TS] bf16."""
            for cht in range(CHT):
                tp = pp.tile([P, NK], rdt, tag="xttp", bufs=2)
                for tt in range(TT):
                    nc.tensor.transpose(
                        tp[:, tt * P:(tt + 1) * P],
                        x_t[:, tt, cht * P:(cht + 1) * P],
                        ident_r[:])
                nc.vector.tensor_copy(xT[:, cht, :], tp[:])

        # ================= phase S: self-attention =================
        def wload(pool, dram, shape, nm):
            t = pool.tile(shape, b16, tag=nm, name=nm)
            nc.sync.dma_start(t[:], dram[:])
            return t

        # reserve the p2 tag early so `small` sits at the stack bottom
        small.tile([P, 2, 2, NK], b16, tag="p2", bufs=2, name="p2")

        pPerm = tc.alloc_tile_pool(name="sbPerm", bufs=1)     # dies at end
        x2_t = pPerm.tile([P, TT, D], rdt, tag="x2", name="x2")
        # lives through both attention cores (written by the self-core filler)
        pK2T = tc.alloc_tile_pool(name="sbK2T", bufs=1)       # dies post-o-proj2
        K2T_t = pK2T.tile([P, CHT, NC], b16, tag="K2T", name="K2T")
        kn2 = [pK2T.tile([P, NT, 2, HD + 1], b16, tag=f"kn2_{c}",
                         name=f"kn2_{c}") for c in range(CHT)]
        ctxT1 = pK2T.tile([P, CHT, TS], b16, tag="ctxT1", name="ctxT1")
        nc.sync.dma_start(mo_t[:], mo_d[:])
        for c in range(CHT):
            for j in range(2):
                nc.vector.tensor_copy(kn2[c][:, :, j, HD:HD + 1], mo_t[:])

        pEnc = tc.alloc_tile_pool(name="sbEnc", bufs=1)       # dies post-self-core
        encT_t = pEnc.tile([P, DT, NC], b16, tag="encT", name="encT")
        w2W_t = pEnc.tile([P, DT, D], b16, tag="w2W", name="w2W")

        pS1 = tc.alloc_tile_pool(name="sbS1", bufs=1)         # dies post-self-core
        QT_t = pS1.tile([P, CHT, NK], b16, tag="QT", name="QT")
        K1T_t = pS1.tile([P, CHT, L], b16, tag="K1T", name="K1T")
        kn1 = [pS1.tile([P, MT, 2, HD + 1], b16, tag=f"kn1_{c}", name=f"kn1_{c}")
               for c in range(CHT)]
        for c in range(CHT):
            nc.vector.memset(kn1[c][:, :, :, HD:HD + 1], 1.0)

        # DMA order is start-latency-critical: xq + q1W first so the Q
        # projection starts ASAP, then w1W + the xb chunks; encT/w2W ride
        # behind the hot stream (first consumed mid-core by the filler).
        pQ1 = tc.alloc_tile_pool(name="sbQ1", bufs=1)         # dies post-QT-proj
        xq_t = pQ1.tile([P, DT, NK], b16, tag="xq", name="xq")
        nc.sync.dma_start(xq_t[:], xqT_d[:])
        q1W_t = wload(pQ1, q1W_d, [P, DT, D], "q1W")
        with tc.tile_pool(name="ps_proj", bufs=1, space="PSUM") as pp:
            proj_T(q1W_t, xq_t, QT_t, pp, "qt", NK)
        pQ1.release()

        pW1 = tc.alloc_tile_pool(name="sbW1", bufs=1)         # dies post-proj
        w1W_t = wload(pW1, w1W_d, [P, DT, D], "w1W")
        pX = tc.alloc_tile_pool(name="sbX", bufs=1)           # dies post-proj
        XBC = 256  # xbT stream-chunk width (2 m-tiles)
        NXB = L // XBC
        with tc.tile_pool(name="ps_proj1b", bufs=1, space="PSUM") as pp:
            for mc in range(NXB):
                xb_t = pX.tile([P, DT, XBC], b16, tag="xb", bufs=2, name="xb")
                nc.sync.dma_start(xb_t[:], xbT_d[:, :, mc * XBC:(mc + 1) * XBC])
                for cht in range(CHT):
                    ps = pp.tile([P, NK], f32, tag="ps_k1", bufs=3)
                    for dti in range(DT):
                        nc.tensor.matmul(ps[:, 0:XBC],
                                         w1W_t[:, dti, cht * P:(cht + 1) * P],
                                         xb_t[:, dti, :],
                                         start=(dti == 0), stop=(dti == DT - 1))
                    if cht % 2 == 0:
                        nc.vector.tensor_copy(
                            K1T_t[:, cht, mc * XBC:(mc + 1) * XBC], ps[:, 0:XBC])
                    else:
                        nc.scalar.copy(
                            K1T_t[:, cht, mc * XBC:(mc + 1) * XBC], ps[:, 0:XBC])
                    if mc == NXB - 1 and cht == 1:
                        # queue the mid-core loads behind the last hot chunks
                        nc.sync.dma_start(encT_t[:], encT_d[:])
                        nc.sync.dma_start(w2W_t[:], w2W_d[:])
                for cht in range(CHT):
                    if cht % 2 == 0:
                        tp = pp.tile([P, NK], b16, tag="kntp", bufs=2)
                    co = (cht % 2) * XBC
                    for j4 in range(2):
                        mt = mc * 2 + j4
                        nc.tensor.transpose(
                            tp[:, co + j4 * P:co + (j4 + 1) * P],
                            K1T_t[:, cht, mt * P:(mt + 1) * P], ident_b[:])
                    knat_copy(kn1, cht, mc * 2, 2, tp[:, co:co + XBC])
        pX.release()
        pW1.release()

        # cross-attention K2^T projection + K-natural build, streamed one
        # matmul at a time into the self core's ACT-bound idle gaps
        def k2_filler(pa):
            for cht in range(CHT):
                for ck in range(NKC):
                    w = min(NK, NC - ck * NK)
                    ps = pa.tile([P, NK], f32, tag="ps_k2", bufs=1,
                                 name="ps_k2")
                    for dti in range(DT):
                        nc.tensor.matmul(
                            ps[:, 0:w],
                            w2W_t[:, dti, cht * P:(cht + 1) * P],
                            encT_t[:, dti, ck * NK:ck * NK + w],
                            start=(dti == 0), stop=(dti == DT - 1))
                        yield w * 0.417
                    nc.vector.tensor_copy(
                        K2T_t[:, cht, ck * NK:ck * NK + w], ps[:, 0:w])
                    yield 0.0
                for g0 in range(0, NT, 4):
                    rem = min(4, NT - g0)
                    tp = pa.tile([P, NK], b16, tag="tpb16", bufs=1)
                    for j4 in range(rem):
                        nc.tensor.transpose(
                            tp[:, j4 * P:(j4 + 1) * P],
                            K2T_t[:, cht, (g0 + j4) * P:(g0 + j4 + 1) * P],
                            ident_b[:])
                        yield 54.0
                    knat_copy(kn2, cht, g0, rem, tp[:, 0:rem * P])
                    yield 0.0

        with tc.tile_pool(name="ps_attn", bufs=1, space="PSUM") as pa:
            attn_core(K1T_t, kn1, QT_t, ctxT1, MT, pa, small,
                      filler=k2_filler(pa), gap_ns=600.0)
        pS1.release()
        pEnc.release()

        # ---- o-proj + LN1 + x1^T + Q2^T ----
        pC2 = tc.alloc_tile_pool(name="sbC2", bufs=1)         # dies post-o-proj2
        x1_t = pC2.tile([P, TT, D], rdt, tag="x1", name="x1")
        pO1ph = tc.alloc_tile_pool(name="sbO1ph", bufs=1)     # dies post-Q2T
        o1W_t = wload(pO1ph, o1W_d, [P, DT, D], "o1W")
        xs_t = pO1ph.tile([P, TT, D], f32, tag="xs", name="xs")
        nc.sync.dma_start(xs_t[:], xs_d.rearrange("(lt p) d -> p lt d", p=P))
        q2W_t = wload(pC2, q2W_d, [P, DT, D], "q2W")
        o2W_t = wload(pC2, o2W_d, [P, DT, D], "o2W")
        rsb1 = pO1ph.tile([P, TT, D], f32, tag="rsb1", name="rsb1")
        with tc.tile_pool(name="ps_o1", bufs=1, space="PSUM") as po:
            o_proj_ln(ctxT1, o1W_t, xs_t, x1_t, "1", po, rsb1)
            x1T_t = pO1ph.tile([P, CHT, TS], b16, tag="x1T", name="x1T")
            transpose_own(x1_t, x1T_t, po)
        Q2T_t = pC2.tile([P, CHT, NK], b16, tag="Q2T", name="Q2T")
        with tc.tile_pool(name="ps_proj2", bufs=1, space="PSUM") as pp:
            proj_T(q2W_t, x1T_t, Q2T_t, pp, "q2t", NK, alt=1)
        pO1ph.release()

        # ================= phase C: cross-attention =================
        ctxT2 = pC2.tile([P, CHT, TS], b16, tag="ctxT2", name="ctxT2")
        with tc.tile_pool(name="ps_attn2", bufs=1, space="PSUM") as pa:
            attn_core(K2T_t, kn2, Q2T_t, ctxT2, NT, pa, small)

        pO2ph = tc.alloc_tile_pool(name="sbO2ph", bufs=1)     # dies post-o-proj2
        rsb2 = pO2ph.tile([P, TT, D], f32, tag="rsb2", name="rsb2")
        with tc.tile_pool(name="ps_o2", bufs=1, space="PSUM") as po:
            o_proj_ln(ctxT2, o2W_t, x1_t, x2_t, "2", po, rsb2)
        pO2ph.release()
        pC2.release()
        pK2T.release()

        # ================= phase F: FFN =================
        pFm = tc.alloc_tile_pool(name="sbFm", bufs=1)         # dies at end
        x2T_t = pFm.tile([P, CHT, TS], b16, tag="x2T", name="x2T")
        pFw1 = tc.alloc_tile_pool(name="sbFw1", bufs=1)       # dies post-ff1
        w1c = []
        for wc in range(4):  # 4 tiles -> ff1 starts after the first quarter
            t = pFw1.tile([P, DT, MLP // 4], b16, tag=f"ffW1_{wc}",
                          name=f"ffW1_{wc}")
            nc.sync.dma_start(t[:], ffW1_d[:, :, wc * D:(wc + 1) * D])
            w1c.append(t)
        with tc.tile_pool(name="ps_x2t", bufs=1, space="PSUM") as po:
            transpose_own(x2_t, x2T_t, po)
        hT_t = pFm.tile([P, HTT, TS], b16, tag="hT", name="hT")
        w2_t = pFm.tile([P, HTT, D], b16, tag="ffW2", name="ffW2")
        with tc.tile_pool(name="ps_ffn", bufs=1, space="PSUM") as pf:
            for ht in range(HTT):
                if ht == HTT // 2:
                    # W1 half consumed; start the W2 fetch mid-ff1
                    nc.sync.dma_start(w2_t[:], ffW2_d[:])
                ps = pf.tile([P, NK], f32, tag="ph", bufs=3)
                wt = w1c[ht // (HTT // 4)]
                ho = (ht % (HTT // 4)) * P
                for dti in range(DT):
                    nc.tensor.matmul(ps[:],
                                     wt[:, dti, ho:ho + P],
                                     x2T_t[:, dti, :],
                                     start=(dti == 0), stop=(dti == DT - 1))
                bias = 0.0 if trivial_ffb else ffb1h_t[:, ht:ht + 1]
                nc.scalar.activation(hT_t[:, ht, :], ps[:], AF.Gelu, bias=bias)
        pFw1.release()
        rsb3 = pFm.tile([P, TT, D], b16, tag="rsb3", name="rsb3")
        if not trivial_ffb:
            ffb2c = pFm.tile([P, D], f32, tag="ffb2", name="ffb2")
            nc.sync.dma_start(ffb2c[:], ffb2b_d[:])
        st3 = small.tile([P, TT, 2, 6], f32, tag="ln_st")
        with tc.tile_pool(name="ps_ff2", bufs=1, space="PSUM") as pf:
            for tt in range(TT):
                pff = pf.tile([P, D], f32, tag="pf2", bufs=2)
                for ht in range(HTT):
                    for dc in range(2):
                        nc.tensor.matmul(
                            pff[:, dc * NK:(dc + 1) * NK],
                            hT_t[:, ht, tt * P:(tt + 1) * P],
                            w2_t[:, ht, dc * NK:(dc + 1) * NK],
                            start=(ht == 0), stop=(ht == HTT - 1))
                nc.vector.tensor_tensor(out=rsb3[:, tt, :], in0=pff[:],
                                        in1=x2_t[:, tt, :], op=ALU.add)
                if not trivial_ffb:
                    nc.vector.tensor_tensor(out=rsb3[:, tt, :],
                                            in0=rsb3[:, tt, :],
                                            in1=ffb2c[:], op=ALU.add)
                ln_stats(rsb3[:, tt, :], st3, tt)

            def store_tt(tt):
                nc.sync.dma_start(out_d[tt * P:(tt + 1) * P, :],
                                  rsb3[:, tt, :])

            ln_finish(rsb3, st3, rsb3, "3", post_tt=store_tt)
        pFm.release()
        pPerm.release()
        small.release()
        cpool.release()

    lp.__exit__(None, None, None)
    nc.compile()
    return nc


def _pmajor(w, p=P):
    """[R, C] row-major -> [p, R//p, C] partition-major tiling."""
    r, c = w.shape
    return np.ascontiguousarray(w.reshape(r // p, p, c).swapaxes(0, 1))


def _host_prep(inputs):
    import ml_dtypes

    b16 = ml_dtypes.bfloat16
    x = np.asarray(inputs["x"], np.float32)          # [B, L, D]
    enc = np.asarray(inputs["enc_output"], np.float32)
    mask = np.asarray(inputs["mask"])                # [B, 1, M, 1]

    n = np.arange(D) // HD
    d = np.arange(D) % HD
    perm = d * H + n

    def pw(q, w, o):
        return (np.asarray(q, np.float32)[:, perm],
                np.asarray(w, np.float32)[:, perm],
                np.asarray(o, np.float32)[perm, :])

    q1W, w1W, o1W = pw(inputs["q1W"], inputs["w1W"], inputs["o1W"])
    q2W, w2W, o2W = pw(inputs["q2W"], inputs["w2W"], inputs["o2W"])
    ffW1 = np.asarray(inputs["ffW1"], np.float32)
    ffW2 = np.asarray(inputs["ffW2"], np.float32)
    ffb1 = np.asarray(inputs["ffb1"], np.float32)
    ffb2 = np.asarray(inputs["ffb2"], np.float32)
    g = {k: np.asarray(inputs[k], np.float32)
         for k in ("g1", "b1", "g2", "b2", "g3", "b3")}

    trivial_affine = all(
        np.all(g[f"g{i}"] == 1.0) and np.all(g[f"b{i}"] == 0.0) for i in (1, 2, 3))
    trivial_ffb = bool(np.all(ffb1 == 0.0) and np.all(ffb2 == 0.0))

    # Compact the cross-attention context per batch: masked positions are
    # dropped (their softmax weight is exactly 0 in the reference); both
    # batches pad to a common m-tile count NT.  Padded slots contribute
    # exactly 0 on-device via zeroed K-natural ones-columns.
    kept = [np.where(~mask[b, 0, :, 0])[0] for b in range(B)]
    NT = max(1, max((len(k) + P - 1) // P for k in kept))
    ncx = NT * P
    encCT = []
    mos = []
    for b in range(B):
        encC = np.zeros((ncx, D), np.float32)
        encC[0:len(kept[b])] = enc[b][kept[b]]
        mo = np.zeros(ncx, np.float32)
        mo[0:len(kept[b])] = 1.0
        encCT.append(np.ascontiguousarray(
            _pmajor(np.ascontiguousarray(encC.T))).astype(b16))
        mos.append(np.ascontiguousarray(
            mo.reshape(NT, P).T.reshape(P, NT, 1)).astype(b16))

    xbT = [np.ascontiguousarray(
        _pmajor(np.ascontiguousarray(x[b].T))).astype(b16) for b in range(B)]

    wmaps = {
        "q1W": _pmajor(q1W).astype(b16), "w1W": _pmajor(w1W).astype(b16),
        "o1W": _pmajor(o1W).astype(b16),
        "q2W": _pmajor(q2W).astype(b16), "w2W": _pmajor(w2W).astype(b16),
        "o2W": _pmajor(o2W).astype(b16),
        "ffW1": _pmajor(ffW1).astype(b16), "ffW2": _pmajor(ffW2).astype(b16),
    }
    if not trivial_affine:
        for k in ("g1", "b1", "g2", "b2", "g3", "b3"):
            wmaps[k + "b"] = np.ascontiguousarray(
                np.broadcast_to(g[k], (P, D)).astype(np.float32))

    in_maps = []
    for r in range(NCORES):
        b = r // (NCORES // B)
        lo = (r % (NCORES // B)) * TS
        xq = np.ascontiguousarray(x[b, lo:lo + TS].T)  # [D, TS]
        im = dict(wmaps)
        im["xqT"] = np.ascontiguousarray(_pmajor(xq)).astype(b16)
        im["xbT"] = xbT[b]
        im["xs"] = np.ascontiguousarray(x[b, lo:lo + TS]).astype(np.float32)
        im["encT"] = encCT[b]
        im["mo"] = mos[b]
        if not trivial_ffb:
            im["ffb2b"] = np.ascontiguousarray(
                np.broadcast_to(ffb2, (P, D)).astype(np.float32))
            im["ffb1h"] = np.ascontiguousarray(
                ffb1.reshape(HTT, P).T.astype(np.float32))
        in_maps.append(im)
    return in_maps, trivial_affine, trivial_ffb, NT


def kernel(**inputs) -> np.ndarray:
    in_maps, trivial_affine, trivial_ffb, NT = _host_prep(inputs)
    key = (trivial_affine, trivial_ffb, NT)
    if key not in _PROGRAM_CACHE:
        _PROGRAM_CACHE[key] = _build_program(*key)
    nc = _PROGRAM_CACHE[key]
    res = run_bass_kernel_spmd(nc, in_maps, list(range(NCORES)))
    out = np.empty((T, D), np.float32)
    for r in range(NCORES):
        out[r * TS:(r + 1) * TS, :] = res.results[r]["out"].astype(np.float32)
    return out.reshape(B, L, D)


# revision 30
# speedup vs baseline: 1.2415x; 1.2415x over previous
"""Trainium2 Bass kernel for a transformer decoder layer — 8-way, zero-collective.

Sharding: pure data-parallel over tokens.  Core r owns rows
[512r, 512(r+1)) of the flattened [B*L, D] = [4096, 1024] token axis
(batch 0 = cores 0-3, batch 1 = cores 4-7).  Weights are fully replicated.

Design notes (vs the tensor-parallel baseline this replaces):
  - In the harness cost model a collective costs 15us + out_bytes/40GBps;
    the TP baseline spent ~1ms of its 1.47ms in AllGather/ReduceScatter.
    Every tensor a core needs besides its own activations is a kernel
    *input* already in HBM, so each core instead recomputes its batch's
    K projections locally (~55us of redundant PE work) and runs the whole
    layer with ZERO collectives.
  - Attention value runs in natural layout: ctx[t, hd] accumulates with
    lhsT = p (exp scores, [m, t]) and rhs = K-natural tiles [m, 64+1];
    the 65th column holds 1 for real context positions and 0 for padded
    ones, so it accumulates the softmax denominator Z AND implements the
    cross-attention mask exactly (pad contributes exp(0)*0 = 0 to both
    numerator and denominator).  No exp bias needed -> exp fuses over two
    m-tiles, halving the ACT access overhead in the softmax stream.
  - The attention cores are ACT(exp)-bound; the PE idle gap under them is
    filled by streaming the cross-attention K2^T projection + K-natural
    build (self core) and the ctx^T transposes (both cores) as filler.
  - PSUM accumulation: start=True marks the whole 2KB zero region, so
    every bank gets exactly one start (first write) and one stop (last).

Numerics: bf16 operands on the PE, fp32 PSUM accumulation, fp32
residual + LayerNorm (bf16 final store).  Host pre-transposes x^T/enc^T,
permutes attention weights head-major, pre-compacts the enc context.
"""

import sys

sys.path.insert(0, "/opt/trn_rl_repo")

import numpy as np

import concourse.bass as bass
import concourse.bacc as bacc
import concourse.mybir as mybir
import concourse.tile as tile
from concourse.bass_utils import run_bass_kernel_spmd
from concourse.masks import make_identity

dt = mybir.dt
AF = mybir.ActivationFunctionType
ALU = mybir.AluOpType

P = 128
D = 1024          # d_model
DT = D // P       # 8 input-channel tiles
H = 16            # heads
HD = 64           # head dim
CHT = D // P      # 8 channel tiles (2 heads each)
MLP = 4096
HTT = MLP // P    # 32 hidden tiles
B, L, M = 2, 2048, 2048
T = B * L
NCORES = 8
TS = T // NCORES  # 512 tokens per core
TT = TS // P      # 4 own-token tiles
NK = 512          # matmul free-dim chunk
MT = L // P       # 16 self-attention m-tiles
EPS = 1e-5

_PROGRAM_CACHE = {}


def _build_program(trivial_affine, trivial_ffb, NT):
    """NT: cross-attention context m-tiles (shared across batches; padded
    slots contribute exactly 0 via zeroed K-natural ones-columns)."""
    NC = NT * P           # cross context tokens (padded)
    NKC = (NC + NK - 1) // NK
    nc = bacc.Bacc(None)
    f32 = dt.float32
    rdt = dt.float32r
    b16 = dt.bfloat16

    def din(name, shape, d):
        return nc.declare_dram_parameter(name, list(shape), d, isOutput=False)

    xqT_d = din("xqT", [P, DT, NK], b16)    # own x^T (pmajor)
    xbT_d = din("xbT", [P, DT, L], b16)     # full-batch x^T (pmajor)
    xs_d = din("xs", [TS, D], f32)          # own x rows (residual)
    encT_d = din("encT", [P, DT, NC], b16)  # compacted enc^T (pmajor)
    mo_d = din("mo", [P, NT, 1], b16)       # 1 kept / 0 padded, per m token
    q1W_d = din("q1W", [P, DT, D], b16)
    w1W_d = din("w1W", [P, DT, D], b16)
    o1W_d = din("o1W", [P, DT, D], b16)
    q2W_d = din("q2W", [P, DT, D], b16)
    w2W_d = din("w2W", [P, DT, D], b16)
    o2W_d = din("o2W", [P, DT, D], b16)
    ffW1_d = din("ffW1", [P, DT, MLP], b16)
    ffW2_d = din("ffW2", [P, HTT, D], b16)
    gb_d = {}
    if not trivial_affine:
        for nm in ("g1", "b1", "g2", "b2", "g3", "b3"):
            gb_d[nm] = din(nm + "b", [P, D], f32)
    if not trivial_ffb:
        ffb2b_d = din("ffb2b", [P, D], f32)
        ffb1h_d = din("ffb1h", [P, HTT], f32)
    out_d = nc.declare_dram_parameter("out", [TS, D], b16, isOutput=True)

    lp = nc.allow_low_precision(reason="bf16 weights/activations")
    lp.__enter__()
    with tile.TileContext(nc) as tc:
        cpool = tc.alloc_tile_pool(name="const", bufs=1)
        small = tc.alloc_tile_pool(name="small", bufs=3)

        ident_f = cpool.tile([P, P], f32)
        make_identity(nc, ident_f[:])
        ident_b = cpool.tile([P, P], b16)
        nc.vector.tensor_copy(ident_b[:], ident_f[:])
        ident_r = cpool.tile([P, P], rdt)
        nc.vector.tensor_copy(ident_r[:], ident_f[:])
        mo_t = cpool.tile([P, NT, 1], b16)
        if not trivial_ffb:
            ffb1h_t = cpool.tile([P, HTT], f32)
            nc.sync.dma_start(ffb1h_t[:], ffb1h_d[:])

        # ---------------- LayerNorm helpers ----------------
        def ln_stats(rsb_tt, st, tt):
            for h in range(2):
                nc.vector.bn_stats(st[:, tt, h, :],
                                   rsb_tt[:, h * NK:(h + 1) * NK])

        def ln_finish(rsb, st, x_out, gkey, post_tt=None):
            mv = small.tile([P, TT, 2], f32, tag="ln_mv")
            for tt in range(TT):
                nc.vector.bn_aggr(mv[:, tt, :], st[:, tt, :, :])
            t = small.tile([P, TT], f32, tag="ln_t")
            nc.vector.tensor_scalar_add(t[:], mv[:, :, 1], EPS)
            s = small.tile([P, TT], f32, tag="ln_s")
            nc.scalar.sqrt(s[:], t[:])
            r0 = small.tile([P, TT], f32, tag="ln_r0")
            nc.vector.reciprocal(r0[:], s[:])
            # one Newton step: r1 = r0 * (1.5 - 0.5 * t * r0^2)
            u = small.tile([P, TT], f32, tag="ln_u")
            nc.vector.tensor_tensor(out=u[:], in0=t[:], in1=r0[:], op=ALU.mult)
            nc.vector.tensor_tensor(out=u[:], in0=u[:], in1=r0[:], op=ALU.mult)
            nc.vector.tensor_scalar(u[:], u[:], -0.5, 1.5, ALU.mult, ALU.add)
            r1 = small.tile([P, TT], f32, tag="ln_r1")
            nc.vector.tensor_tensor(out=r1[:], in0=r0[:], in1=u[:], op=ALU.mult)
            for tt in range(TT):
                if trivial_affine:
                    nc.vector.tensor_scalar(
                        x_out[:, tt, :], rsb[:, tt, :], mv[:, tt, 0:1],
                        r1[:, tt:tt + 1], ALU.subtract, ALU.mult)
                else:
                    g_t = small.tile([P, D], f32, tag="ln_g", bufs=2)
                    nc.sync.dma_start(g_t[:], gb_d["g" + gkey][:])
                    b_t = small.tile([P, D], f32, tag="ln_b", bufs=2)
                    nc.sync.dma_start(b_t[:], gb_d["b" + gkey][:])
                    nc.vector.tensor_scalar(
                        rsb[:, tt, :], rsb[:, tt, :], mv[:, tt, 0:1],
                        r1[:, tt:tt + 1], ALU.subtract, ALU.mult)
                    nc.vector.tensor_tensor(out=rsb[:, tt, :], in0=rsb[:, tt, :],
                                            in1=g_t[:], op=ALU.mult)
                    nc.vector.tensor_tensor(out=x_out[:, tt, :], in0=rsb[:, tt, :],
                                            in1=b_t[:], op=ALU.add)
                if post_tt is not None:
                    post_tt(tt)

        # ---------------- attention building blocks ----------------
        def proj_T(wt, rhs_t, dst, pp, nm, width, alt=0):
            """dst[:, cht, 0:width] = (W^T x^T) bf16 for all channel tiles."""
            for cht in range(CHT):
                ps = pp.tile([P, NK], f32, tag=f"ps_{nm}", bufs=3)
                for dti in range(DT):
                    nc.tensor.matmul(ps[:, 0:width],
                                     wt[:, dti, cht * P:(cht + 1) * P],
                                     rhs_t[:, dti, 0:width],
                                     start=(dti == 0), stop=(dti == DT - 1))
                if (cht + alt) % 2 == 0:
                    nc.vector.tensor_copy(dst[:, cht, 0:width], ps[:, 0:width])
                else:
                    nc.scalar.copy(dst[:, cht, 0:width], ps[:, 0:width])

        def knat_copy(kn_t, cht, g0, rem, src):
            """src: [P, rem*P] AP of transposed K tiles (psum)."""
            nc.vector.tensor_copy(
                kn_t[cht][:, g0:g0 + rem, :, 0:HD],
                src.rearrange("p (mt hd) -> p mt hd", hd=P)
                .rearrange("p mt (h c) -> p mt h c", c=HD))

        def attn_core(KT_t, kn_t, QT_t, ctxT, n_mt, pa, pat,
                      filler=None, gap_ns=600.0):
            """Scores + fused-exp softmax + natural-layout value for own 512
            queries, all 16 heads; normalized context transposed into
            ctxT [P, CHT, TS] per head-pair.  filler: generator emitting one
            PE work unit per next(), yielding its ns cost — pulled into the
            per-pair ACT-bound idle gap."""
            credit = 0.0
            for hp in range(CHT):
                ctxp = [pa.tile([P, TT, P], f32, tag=f"ctx{j}", bufs=1,
                                name=f"ctx{j}") for j in range(2)]
                for mt in range(n_mt):
                    s2 = pa.tile([P, 2, NK], f32, tag="s2", bufs=2)
                    for j in range(2):
                        nc.tensor.matmul(
                            s2[:, j, :],
                            KT_t[j * HD:(j + 1) * HD, hp, mt * P:(mt + 1) * P],
                            QT_t[j * HD:(j + 1) * HD, hp, :],
                            start=True, stop=True)
                    p2 = pat.tile([P, 2, NK], b16, tag="p2", bufs=2, name="p2")
                    nc.scalar.activation(p2[:], s2[:], AF.Exp, scale=0.125)
                    for j in range(2):
                        for ts4 in range(TT):
                            # one start/stop per 2KB PSUM zero region
                            nc.tensor.matmul(
                                ctxp[j][:, ts4, 0:HD + 1],
                                p2[:, j, ts4 * P:(ts4 + 1) * P],
                                kn_t[hp][:, mt, j, :],
                                start=(mt == 0 and ts4 == 0),
                                stop=(mt == n_mt - 1 and ts4 == TT - 1))
                    if filler is not None:
                        credit += gap_ns
                        while credit > 0:
                            c = next(filler, None)
                            if c is None:
                                filler = None
                                break
                            credit -= c
                rec = small.tile([P, 2, TT, 1], f32, tag="rec")
                for j in range(2):
                    nc.vector.reciprocal(rec[:, j], ctxp[j][:, :, HD:HD + 1])
                ctxh = pat.tile([P, TT, P], b16, tag="ctxh", bufs=2,
                                name="ctxh")
                for j in range(2):
                    for ts4 in range(TT):
                        nc.vector.tensor_scalar(
                            ctxh[:, ts4, j * HD:(j + 1) * HD],
                            ctxp[j][:, ts4, 0:HD],
                            rec[:, j, ts4], None, ALU.mult)
                tp = pa.tile([P, NK], b16, tag="tpb16", bufs=1)
                for ts4 in range(TT):
                    nc.tensor.transpose(tp[:, ts4 * P:(ts4 + 1) * P],
                                        ctxh[:, ts4, :], ident_b[:])
                nc.vector.tensor_copy(ctxT[:, hp, :], tp[:])
            if filler is not None:
                while next(filler, None) is not None:
                    pass

        def o_proj_ln(ctxT, ow_t, resid, x_out, gkey, pp, rsb, extra=None,
                      post_tt=None):
            st = small.tile([P, TT, 2, 6], f32, tag="ln_st")
            for tt in range(TT):
                po = pp.tile([P, D], f32, tag="po", bufs=2)
                for cht in range(CHT):
                    for dc in range(2):
                        nc.tensor.matmul(
                            po[:, dc * NK:(dc + 1) * NK],
                            ctxT[:, cht, tt * P:(tt + 1) * P],
                            ow_t[:, cht, dc * NK:(dc + 1) * NK],
                            start=(cht == 0), stop=(cht == CHT - 1))
                nc.vector.tensor_tensor(out=rsb[:, tt, :], in0=po[:],
                                        in1=resid[:, tt, :], op=ALU.add)
                if extra is not None:
                    nc.vector.tensor_tensor(out=rsb[:, tt, :], in0=rsb[:, tt, :],
                                            in1=extra[:], op=ALU.add)
                ln_stats(rsb[:, tt, :], st, tt)
            ln_finish(rsb, st, x_out, gkey, post_tt=post_tt)

        def transpose_own(x_t, xT, pp):
            """x_t [P, TT, D] (f32r) -> xT [P, CHT, TS] bf16."""
            for cht in range(CHT):
                tp = pp.tile([P, NK], rdt, tag="xttp", bufs=2)
                for tt in range(TT):
                    nc.tensor.transpose(
                        tp[:, tt * P:(tt + 1) * P],
                        x_t[:, tt, cht * P:(cht + 1) * P],
                        ident_r[:])
                nc.vector.tensor_copy(xT[:, cht, :], tp[:])

        # ================= phase S: self-attention =================
        def wload(pool, dram, shape, nm):
            t = pool.tile(shape, b16, tag=nm, name=nm)
            nc.sync.dma_start(t[:], dram[:])
            return t

        # reserve the p2 tag early so `small` sits at the stack bottom
        small.tile([P, 2, NK], b16, tag="p2", bufs=2, name="p2")

        pPerm = tc.alloc_tile_pool(name="sbPerm", bufs=1)     # dies at end
        x2_t = pPerm.tile([P, TT, D], rdt, tag="x2", name="x2")
        # lives through both attention cores (written by the self-core filler)
        pK2T = tc.alloc_tile_pool(name="sbK2T", bufs=1)       # dies post-o-proj2
        K2T_t = pK2T.tile([P, CHT, NC], b16, tag="K2T", name="K2T")
        kn2 = [pK2T.tile([P, NT, 2, HD + 1], b16, tag=f"kn2_{c}",
                         name=f"kn2_{c}") for c in range(CHT)]
        ctxT1 = pK2T.tile([P, CHT, TS], b16, tag="ctxT1", name="ctxT1")
        nc.sync.dma_start(mo_t[:], mo_d[:])
        for c in range(CHT):
            for j in range(2):
                nc.vector.tensor_copy(kn2[c][:, :, j, HD:HD + 1], mo_t[:])

        pEnc = tc.alloc_tile_pool(name="sbEnc", bufs=1)       # dies post-self-core
        encT_t = pEnc.tile([P, DT, NC], b16, tag="encT", name="encT")
        w2W_t = pEnc.tile([P, DT, D], b16, tag="w2W", name="w2W")

        pS1 = tc.alloc_tile_pool(name="sbS1", bufs=1)         # dies post-self-core
        QT_t = pS1.tile([P, CHT, NK], b16, tag="QT", name="QT")
        K1T_t = pS1.tile([P, CHT, L], b16, tag="K1T", name="K1T")
        kn1 = [pS1.tile([P, MT, 2, HD + 1], b16, tag=f"kn1_{c}", name=f"kn1_{c}")
               for c in range(CHT)]
        for c in range(CHT):
            nc.vector.memset(kn1[c][:, :, :, HD:HD + 1], 1.0)

        # DMA order is start-latency-critical: xq + q1W first so the Q
        # projection starts ASAP, then w1W + the xb chunks; encT/w2W ride
        # behind the hot stream (first consumed mid-core by the filler).
        pQ1 = tc.alloc_tile_pool(name="sbQ1", bufs=1)         # dies post-QT-proj
        xq_t = pQ1.tile([P, DT, NK], b16, tag="xq", name="xq")
        nc.sync.dma_start(xq_t[:], xqT_d[:])
        q1W_t = wload(pQ1, q1W_d, [P, DT, D], "q1W")
        with tc.tile_pool(name="ps_proj", bufs=1, space="PSUM") as pp:
            proj_T(q1W_t, xq_t, QT_t, pp, "qt", NK)
        pQ1.release()

        pW1 = tc.alloc_tile_pool(name="sbW1", bufs=1)         # dies post-proj
        w1W_t = wload(pW1, w1W_d, [P, DT, D], "w1W")
        pX = tc.alloc_tile_pool(name="sbX", bufs=1)           # dies post-proj
        XBC = 256  # xbT stream-chunk width (2 m-tiles)
        NXB = L // XBC
        with tc.tile_pool(name="ps_proj1b", bufs=1, space="PSUM") as pp:
            for mc in range(NXB):
                xb_t = pX.tile([P, DT, XBC], b16, tag="xb", bufs=2, name="xb")
                nc.sync.dma_start(xb_t[:], xbT_d[:, :, mc * XBC:(mc + 1) * XBC])
                for cht in range(CHT):
                    ps = pp.tile([P, NK], f32, tag="ps_k1", bufs=3)
                    for dti in range(DT):
                        nc.tensor.matmul(ps[:, 0:XBC],
                                         w1W_t[:, dti, cht * P:(cht + 1) * P],
                                         xb_t[:, dti, :],
                                         start=(dti == 0), stop=(dti == DT - 1))
                    if cht % 2 == 0:
                        nc.vector.tensor_copy(
                            K1T_t[:, cht, mc * XBC:(mc + 1) * XBC], ps[:, 0:XBC])
                    else:
                        nc.scalar.copy(
                            K1T_t[:, cht, mc * XBC:(mc + 1) * XBC], ps[:, 0:XBC])
                    if mc == NXB - 1 and cht == 1:
                        # queue the mid-core loads behind the last hot chunks
                        nc.sync.dma_start(encT_t[:], encT_d[:])
                        nc.sync.dma_start(w2W_t[:], w2W_d[:])
                for cht in range(CHT):
                    if cht % 2 == 0:
                        tp = pp.tile([P, NK], b16, tag="kntp", bufs=2)
                    co = (cht % 2) * XBC
                    for j4 in range(2):
                        mt = mc * 2 + j4
                        nc.tensor.transpose(
                            tp[:, co + j4 * P:co + (j4 + 1) * P],
                            K1T_t[:, cht, mt * P:(mt + 1) * P], ident_b[:])
                    knat_copy(kn1, cht, mc * 2, 2, tp[:, co:co + XBC])
        pX.release()
        pW1.release()

        # cross-attention K2^T projection + K-natural build, streamed one
        # matmul at a time into the self core's ACT-bound idle gaps
        def k2_filler(pa):
            for cht in range(CHT):
                for ck in range(NKC):
                    w = min(NK, NC - ck * NK)
                    ps = pa.tile([P, NK], f32, tag="ps_k2", bufs=1,
                                 name="ps_k2")
                    for dti in range(DT):
                        nc.tensor.matmul(
                            ps[:, 0:w],
                            w2W_t[:, dti, cht * P:(cht + 1) * P],
                            encT_t[:, dti, ck * NK:ck * NK + w],
                            start=(dti == 0), stop=(dti == DT - 1))
                        yield w * 0.417
                    nc.vector.tensor_copy(
                        K2T_t[:, cht, ck * NK:ck * NK + w], ps[:, 0:w])
                    yield 0.0
                for g0 in range(0, NT, 4):
                    rem = min(4, NT - g0)
                    tp = pa.tile([P, NK], b16, tag="tpb16", bufs=1)
                    for j4 in range(rem):
                        nc.tensor.transpose(
                            tp[:, j4 * P:(j4 + 1) * P],
                            K2T_t[:, cht, (g0 + j4) * P:(g0 + j4 + 1) * P],
                            ident_b[:])
                        yield 54.0
                    knat_copy(kn2, cht, g0, rem, tp[:, 0:rem * P])
                    yield 0.0

        with tc.tile_pool(name="ps_attn", bufs=1, space="PSUM") as pa:
            attn_core(K1T_t, kn1, QT_t, ctxT1, MT, pa, small,
                      filler=k2_filler(pa), gap_ns=370.0)
        pS1.release()
        pEnc.release()

        # ---- o-proj + LN1 + x1^T + Q2^T ----
        pC2 = tc.alloc_tile_pool(name="sbC2", bufs=1)         # dies post-o-proj2
        x1_t = pC2.tile([P, TT, D], rdt, tag="x1", name="x1")
        pO1ph = tc.alloc_tile_pool(name="sbO1ph", bufs=1)     # dies post-Q2T
        o1W_t = wload(pO1ph, o1W_d, [P, DT, D], "o1W")
        xs_t = pO1ph.tile([P, TT, D], f32, tag="xs", name="xs")
        nc.sync.dma_start(xs_t[:], xs_d.rearrange("(lt p) d -> p lt d", p=P))
        q2W_t = wload(pC2, q2W_d, [P, DT, D], "q2W")
        o2W_t = wload(pC2, o2W_d, [P, DT, D], "o2W")
        rsb1 = pO1ph.tile([P, TT, D], f32, tag="rsb1", name="rsb1")
        with tc.tile_pool(name="ps_o1", bufs=1, space="PSUM") as po:
            o_proj_ln(ctxT1, o1W_t, xs_t, x1_t, "1", po, rsb1)
            x1T_t = pO1ph.tile([P, CHT, TS], b16, tag="x1T", name="x1T")
            transpose_own(x1_t, x1T_t, po)
        Q2T_t = pC2.tile([P, CHT, NK], b16, tag="Q2T", name="Q2T")
        with tc.tile_pool(name="ps_proj2", bufs=1, space="PSUM") as pp:
            proj_T(q2W_t, x1T_t, Q2T_t, pp, "q2t", NK, alt=1)
        pO1ph.release()

        # ================= phase C: cross-attention =================
        ctxT2 = pC2.tile([P, CHT, TS], b16, tag="ctxT2", name="ctxT2")
        with tc.tile_pool(name="ps_attn2", bufs=1, space="PSUM") as pa:
            attn_core(K2T_t, kn2, Q2T_t, ctxT2, NT, pa, small)

        pO2ph = tc.alloc_tile_pool(name="sbO2ph", bufs=1)     # dies post-o-proj2
        rsb2 = pO2ph.tile([P, TT, D], f32, tag="rsb2", name="rsb2")
        with tc.tile_pool(name="ps_o2", bufs=1, space="PSUM") as po:
            o_proj_ln(ctxT2, o2W_t, x1_t, x2_t, "2", po, rsb2)
        pO2ph.release()
        pC2.release()
        pK2T.release()

        # ================= phase F: FFN =================
        pFm = tc.alloc_tile_pool(name="sbFm", bufs=1)         # dies at end
        x2T_t = pFm.tile([P, CHT, TS], b16, tag="x2T", name="x2T")
        pFw1 = tc.alloc_tile_pool(name="sbFw1", bufs=1)       # dies post-ff1
        w1c = []
        for wc in range(4):  # 4 tiles -> ff1 starts after the first quarter
            t = pFw1.tile([P, DT, MLP // 4], b16, tag=f"ffW1_{wc}",
                          name=f"ffW1_{wc}")
            nc.sync.dma_start(t[:], ffW1_d[:, :, wc * D:(wc + 1) * D])
            w1c.append(t)
        with tc.tile_pool(name="ps_x2t", bufs=1, space="PSUM") as po:
            transpose_own(x2_t, x2T_t, po)
        hT_t = pFm.tile([P, HTT, TS], b16, tag="hT", name="hT")
        w2_t = pFm.tile([P, HTT, D], b16, tag="ffW2", name="ffW2")
        with tc.tile_pool(name="ps_ffn", bufs=1, space="PSUM") as pf:
            for ht in range(HTT):
                if ht == HTT // 2:
                    # W1 half consumed; start the W2 fetch mid-ff1
                    nc.sync.dma_start(w2_t[:], ffW2_d[:])
                ps = pf.tile([P, NK], f32, tag="ph", bufs=3)
                wt = w1c[ht // (HTT // 4)]
                ho = (ht % (HTT // 4)) * P
                for dti in range(DT):
                    nc.tensor.matmul(ps[:],
                                     wt[:, dti, ho:ho + P],
                                     x2T_t[:, dti, :],
                                     start=(dti == 0), stop=(dti == DT - 1))
                bias = 0.0 if trivial_ffb else ffb1h_t[:, ht:ht + 1]
                nc.scalar.activation(hT_t[:, ht, :], ps[:], AF.Gelu, bias=bias)
        pFw1.release()
        rsb3 = pFm.tile([P, TT, D], b16, tag="rsb3", name="rsb3")
        if not trivial_ffb:
            ffb2c = pFm.tile([P, D], f32, tag="ffb2", name="ffb2")
            nc.sync.dma_start(ffb2c[:], ffb2b_d[:])
        st3 = small.tile([P, TT, 2, 6], f32, tag="ln_st")
        with tc.tile_pool(name="ps_ff2", bufs=1, space="PSUM") as pf:
            for tt in range(TT):
                pff = pf.tile([P, D], f32, tag="pf2", bufs=2)
                for ht in range(HTT):
                    for dc in range(2):
                        nc.tensor.matmul(
                            pff[:, dc * NK:(dc + 1) * NK],
                            hT_t[:, ht, tt * P:(tt + 1) * P],
                            w2_t[:, ht, dc * NK:(dc + 1) * NK],
                            start=(ht == 0), stop=(ht == HTT - 1))
                nc.vector.tensor_tensor(out=rsb3[:, tt, :], in0=pff[:],
                                        in1=x2_t[:, tt, :], op=ALU.add)
                if not trivial_ffb:
                    nc.vector.tensor_tensor(out=rsb3[:, tt, :],
                                            in0=rsb3[:, tt, :],
                                            in1=ffb2c[:], op=ALU.add)
                ln_stats(rsb3[:, tt, :], st3, tt)

            def store_tt(tt):
                nc.sync.dma_start(out_d[tt * P:(tt + 1) * P, :],
                                  rsb3[:, tt, :])

            ln_finish(rsb3, st3, rsb3, "3", post_tt=store_tt)
        pFm.release()
        pPerm.release()
        small.release()
        cpool.release()

    lp.__exit__(None, None, None)
    nc.compile()
    return nc


def _pmajor(w, p=P):
    """[R, C] row-major -> [p, R//p, C] partition-major tiling."""
    r, c = w.shape
    return np.ascontiguousarray(w.reshape(r // p, p, c).swapaxes(0, 1))


def _host_prep(inputs):
    import ml_dtypes

    b16 = ml_dtypes.bfloat16
    x = np.asarray(inputs["x"], np.float32)          # [B, L, D]
    enc = np.asarray(inputs["enc_output"], np.float32)
    mask = np.asarray(inputs["mask"])                # [B, 1, M, 1]

    n = np.arange(D) // HD
    d = np.arange(D) % HD
    perm = d * H + n

    def pw(q, w, o):
        return (np.asarray(q, np.float32)[:, perm],
                np.asarray(w, np.float32)[:, perm],
                np.asarray(o, np.float32)[perm, :])

    q1W, w1W, o1W = pw(inputs["q1W"], inputs["w1W"], inputs["o1W"])
    q2W, w2W, o2W = pw(inputs["q2W"], inputs["w2W"], inputs["o2W"])
    ffW1 = np.asarray(inputs["ffW1"], np.float32)
    ffW2 = np.asarray(inputs["ffW2"], np.float32)
    ffb1 = np.asarray(inputs["ffb1"], np.float32)
    ffb2 = np.asarray(inputs["ffb2"], np.float32)
    g = {k: np.asarray(inputs[k], np.float32)
         for k in ("g1", "b1", "g2", "b2", "g3", "b3")}

    trivial_affine = all(
        np.all(g[f"g{i}"] == 1.0) and np.all(g[f"b{i}"] == 0.0) for i in (1, 2, 3))
    trivial_ffb = bool(np.all(ffb1 == 0.0) and np.all(ffb2 == 0.0))

    # Compact the cross-attention context per batch: masked positions are
    # dropped (their softmax weight is exactly 0 in the reference); both
    # batches pad to a common m-tile count NT.  Padded slots contribute
    # exactly 0 on-device via zeroed K-natural ones-columns.
    kept = [np.where(~mask[b, 0, :, 0])[0] for b in range(B)]
    NT = max(1, max((len(k) + P - 1) // P for k in kept))
    ncx = NT * P
    encCT = []
    mos = []
    for b in range(B):
        encC = np.zeros((ncx, D), np.float32)
        encC[0:len(kept[b])] = enc[b][kept[b]]
        mo = np.zeros(ncx, np.float32)
        mo[0:len(kept[b])] = 1.0
        encCT.append(np.ascontiguousarray(
            _pmajor(np.ascontiguousarray(encC.T))).astype(b16))
        mos.append(np.ascontiguousarray(
            mo.reshape(NT, P).T.reshape(P, NT, 1)).astype(b16))

    xbT = [np.ascontiguousarray(
        _pmajor(np.ascontiguousarray(x[b].T))).astype(b16) for b in range(B)]

    wmaps = {
        "q1W": _pmajor(q1W).astype(b16), "w1W": _pmajor(w1W).astype(b16),
        "o1W": _pmajor(o1W).astype(b16),
        "q2W": _pmajor(q2W).astype(b16), "w2W": _pmajor(w2W).astype(b16),
        "o2W": _pmajor(o2W).astype(b16),
        "ffW1": _pmajor(ffW1).astype(b16), "ffW2": _pmajor(ffW2).astype(b16),
    }
    if not trivial_affine:
        for k in ("g1", "b1", "g2", "b2", "g3", "b3"):
            wmaps[k + "b"] = np.ascontiguousarray(
                np.broadcast_to(g[k], (P, D)).astype(np.float32))

    in_maps = []
    for r in range(NCORES):
        b = r // (NCORES // B)
        lo = (r % (NCORES // B)) * TS
        xq = np.ascontiguousarray(x[b, lo:lo + TS].T)  # [D, TS]
        im = dict(wmaps)
        im["xqT"] = np.ascontiguousarray(_pmajor(xq)).astype(b16)
        im["xbT"] = xbT[b]
        im["xs"] = np.ascontiguousarray(x[b, lo:lo + TS]).astype(np.float32)
        im["encT"] = encCT[b]
        im["mo"] = mos[b]
        if not trivial_ffb:
            im["ffb2b"] = np.ascontiguousarray(
                np.broadcast_to(ffb2, (P, D)).astype(np.float32))
            im["ffb1h"] = np.ascontiguousarray(
                ffb1.reshape(HTT, P).T.astype(np.float32))
        in_maps.append(im)
    return in_maps, trivial_affine, trivial_ffb, NT


def kernel(**inputs) -> np.ndarray:
    in_maps, trivial_affine, trivial_ffb, NT = _host_prep(inputs)
    key = (trivial_affine, trivial_ffb, NT)
    if key not in _PROGRAM_CACHE:
        _PROGRAM_CACHE[key] = _build_program(*key)
    nc = _PROGRAM_CACHE[key]
    res = run_bass_kernel_spmd(nc, in_maps, list(range(NCORES)))
    out = np.empty((T, D), np.float32)
    for r in range(NCORES):
        out[r * TS:(r + 1) * TS, :] = res.results[r]["out"].astype(np.float32)
    return out.reshape(B, L, D)


# revision 32
# speedup vs baseline: 1.2568x; 1.0123x over previous
"""Trainium2 Bass kernel for a transformer decoder layer — 8-way, zero-collective.

Sharding: pure data-parallel over tokens.  Core r owns rows
[512r, 512(r+1)) of the flattened [B*L, D] = [4096, 1024] token axis
(batch 0 = cores 0-3, batch 1 = cores 4-7).  Weights are fully replicated.

Design notes (vs the tensor-parallel baseline this replaces):
  - In the harness cost model a collective costs 15us + out_bytes/40GBps;
    the TP baseline spent ~1ms of its 1.47ms in AllGather/ReduceScatter.
    Every tensor a core needs besides its own activations is a kernel
    *input* already in HBM, so each core instead recomputes its batch's
    K projections locally (~55us of redundant PE work) and runs the whole
    layer with ZERO collectives.
  - Attention value runs in natural layout: ctx[t, hd] accumulates with
    lhsT = p (exp scores, [m, t]) and rhs = K-natural tiles [m, 64+1];
    the 65th column holds 1 for real context positions and 0 for padded
    ones, so it accumulates the softmax denominator Z AND implements the
    cross-attention mask exactly (pad contributes exp(0)*0 = 0 to both
    numerator and denominator).  No exp bias needed -> exp fuses over two
    m-tiles, halving the ACT access overhead in the softmax stream.
  - The attention cores are ACT(exp)-bound; the PE idle gap under them is
    filled by streaming the cross-attention K2^T projection + K-natural
    build (self core) and the ctx^T transposes (both cores) as filler.
  - PSUM accumulation: start=True marks the whole 2KB zero region, so
    every bank gets exactly one start (first write) and one stop (last).

Numerics: bf16 operands on the PE, fp32 PSUM accumulation, fp32
residual + LayerNorm (bf16 final store).  Host pre-transposes x^T/enc^T,
permutes attention weights head-major, pre-compacts the enc context.
"""

import sys

sys.path.insert(0, "/opt/trn_rl_repo")

import numpy as np

import concourse.bass as bass
import concourse.bacc as bacc
import concourse.mybir as mybir
import concourse.tile as tile
from concourse.bass_utils import run_bass_kernel_spmd
from concourse.masks import make_identity

dt = mybir.dt
AF = mybir.ActivationFunctionType
ALU = mybir.AluOpType

P = 128
D = 1024          # d_model
DT = D // P       # 8 input-channel tiles
H = 16            # heads
HD = 64           # head dim
CHT = D // P      # 8 channel tiles (2 heads each)
MLP = 4096
HTT = MLP // P    # 32 hidden tiles
B, L, M = 2, 2048, 2048
T = B * L
NCORES = 8
TS = T // NCORES  # 512 tokens per core
TT = TS // P      # 4 own-token tiles
NK = 512          # matmul free-dim chunk
MT = L // P       # 16 self-attention m-tiles
EPS = 1e-5

_PROGRAM_CACHE = {}


def _build_program(trivial_affine, trivial_ffb, NT):
    """NT: cross-attention context m-tiles (shared across batches; padded
    slots contribute exactly 0 via zeroed K-natural ones-columns)."""
    NC = NT * P           # cross context tokens (padded)
    NKC = (NC + NK - 1) // NK
    nc = bacc.Bacc(None)
    f32 = dt.float32
    rdt = dt.float32r
    b16 = dt.bfloat16

    def din(name, shape, d):
        return nc.declare_dram_parameter(name, list(shape), d, isOutput=False)

    xqT_d = din("xqT", [P, DT, NK], b16)    # own x^T (pmajor)
    xbT_d = din("xbT", [P, DT, L], b16)     # full-batch x^T (pmajor)
    xs_d = din("xs", [TS, D], f32)          # own x rows (residual)
    encT_d = din("encT", [P, DT, NC], b16)  # compacted enc^T (pmajor)
    mo_d = din("mo", [P, NT, 1], b16)       # 1 kept / 0 padded, per m token
    q1W_d = din("q1W", [P, DT, D], b16)
    w1W_d = din("w1W", [P, DT, D], b16)
    o1W_d = din("o1W", [P, DT, D], b16)
    q2W_d = din("q2W", [P, DT, D], b16)
    w2W_d = din("w2W", [P, DT, D], b16)
    o2W_d = din("o2W", [P, DT, D], b16)
    ffW1_d = din("ffW1", [P, DT, MLP], b16)
    ffW2_d = din("ffW2", [P, HTT, D], b16)
    gb_d = {}
    if not trivial_affine:
        for nm in ("g1", "b1", "g2", "b2", "g3", "b3"):
            gb_d[nm] = din(nm + "b", [P, D], f32)
    if not trivial_ffb:
        ffb2b_d = din("ffb2b", [P, D], f32)
        ffb1h_d = din("ffb1h", [P, HTT], f32)
    out_d = nc.declare_dram_parameter("out", [TS, D], b16, isOutput=True)

    lp = nc.allow_low_precision(reason="bf16 weights/activations")
    lp.__enter__()
    with tile.TileContext(nc) as tc:
        cpool = tc.alloc_tile_pool(name="const", bufs=1)
        small = tc.alloc_tile_pool(name="small", bufs=3)

        ident_f = cpool.tile([P, P], f32)
        make_identity(nc, ident_f[:])
        ident_b = cpool.tile([P, P], b16)
        nc.vector.tensor_copy(ident_b[:], ident_f[:])
        ident_r = cpool.tile([P, P], rdt)
        nc.vector.tensor_copy(ident_r[:], ident_f[:])
        mo_t = cpool.tile([P, NT, 1], b16)
        if not trivial_ffb:
            ffb1h_t = cpool.tile([P, HTT], f32)
            nc.sync.dma_start(ffb1h_t[:], ffb1h_d[:])

        # ---------------- LayerNorm helpers ----------------
        def ln_stats(rsb_tt, st, tt):
            for h in range(2):
                nc.vector.bn_stats(st[:, tt, h, :],
                                   rsb_tt[:, h * NK:(h + 1) * NK])

        def ln_finish(rsb, st, x_out, gkey, post_tt=None):
            mv = small.tile([P, TT, 2], f32, tag="ln_mv")
            for tt in range(TT):
                nc.vector.bn_aggr(mv[:, tt, :], st[:, tt, :, :])
            t = small.tile([P, TT], f32, tag="ln_t")
            nc.vector.tensor_scalar_add(t[:], mv[:, :, 1], EPS)
            s = small.tile([P, TT], f32, tag="ln_s")
            nc.scalar.sqrt(s[:], t[:])
            r0 = small.tile([P, TT], f32, tag="ln_r0")
            nc.vector.reciprocal(r0[:], s[:])
            # one Newton step: r1 = r0 * (1.5 - 0.5 * t * r0^2)
            u = small.tile([P, TT], f32, tag="ln_u")
            nc.vector.tensor_tensor(out=u[:], in0=t[:], in1=r0[:], op=ALU.mult)
            nc.vector.tensor_tensor(out=u[:], in0=u[:], in1=r0[:], op=ALU.mult)
            nc.vector.tensor_scalar(u[:], u[:], -0.5, 1.5, ALU.mult, ALU.add)
            r1 = small.tile([P, TT], f32, tag="ln_r1")
            nc.vector.tensor_tensor(out=r1[:], in0=r0[:], in1=u[:], op=ALU.mult)
            for tt in range(TT):
                if trivial_affine:
                    nc.vector.tensor_scalar(
                        x_out[:, tt, :], rsb[:, tt, :], mv[:, tt, 0:1],
                        r1[:, tt:tt + 1], ALU.subtract, ALU.mult)
                else:
                    g_t = small.tile([P, D], f32, tag="ln_g", bufs=2)
                    nc.sync.dma_start(g_t[:], gb_d["g" + gkey][:])
                    b_t = small.tile([P, D], f32, tag="ln_b", bufs=2)
                    nc.sync.dma_start(b_t[:], gb_d["b" + gkey][:])
                    nc.vector.tensor_scalar(
                        rsb[:, tt, :], rsb[:, tt, :], mv[:, tt, 0:1],
                        r1[:, tt:tt + 1], ALU.subtract, ALU.mult)
                    nc.vector.tensor_tensor(out=rsb[:, tt, :], in0=rsb[:, tt, :],
                                            in1=g_t[:], op=ALU.mult)
                    nc.vector.tensor_tensor(out=x_out[:, tt, :], in0=rsb[:, tt, :],
                                            in1=b_t[:], op=ALU.add)
                if post_tt is not None:
                    post_tt(tt)

        # ---------------- attention building blocks ----------------
        def proj_T(wt, rhs_t, dst, pp, nm, width, alt=0):
            """dst[:, cht, 0:width] = (W^T x^T) bf16 for all channel tiles."""
            for cht in range(CHT):
                ps = pp.tile([P, NK], f32, tag=f"ps_{nm}", bufs=3)
                for dti in range(DT):
                    nc.tensor.matmul(ps[:, 0:width],
                                     wt[:, dti, cht * P:(cht + 1) * P],
                                     rhs_t[:, dti, 0:width],
                                     start=(dti == 0), stop=(dti == DT - 1))
                if (cht + alt) % 2 == 0:
                    nc.vector.tensor_copy(dst[:, cht, 0:width], ps[:, 0:width])
                else:
                    nc.scalar.copy(dst[:, cht, 0:width], ps[:, 0:width])

        def knat_copy(kn_t, cht, g0, rem, src):
            """src: [P, rem*P] AP of transposed K tiles (psum)."""
            nc.vector.tensor_copy(
                kn_t[cht][:, g0:g0 + rem, :, 0:HD],
                src.rearrange("p (mt hd) -> p mt hd", hd=P)
                .rearrange("p mt (h c) -> p mt h c", c=HD))

        def attn_core(KT_t, kn_t, QT_t, ctxT, n_mt, pa, pat,
                      filler=None, gap_ns=600.0):
            """Scores + fused-exp softmax + natural-layout value for own 512
            queries, all 16 heads; normalized context transposed into
            ctxT [P, CHT, TS] per head-pair.  filler: generator emitting one
            PE work unit per next(), yielding its ns cost — pulled into the
            per-pair ACT-bound idle gap."""
            credit = 0.0
            for hp in range(CHT):
                ctxp = [pa.tile([P, TT, P], f32, tag=f"ctx{j}", bufs=1,
                                name=f"ctx{j}") for j in range(2)]
                for mt in range(n_mt):
                    s2 = pa.tile([P, 2, NK], f32, tag="s2", bufs=2)
                    for j in range(2):
                        nc.tensor.matmul(
                            s2[:, j, :],
                            KT_t[j * HD:(j + 1) * HD, hp, mt * P:(mt + 1) * P],
                            QT_t[j * HD:(j + 1) * HD, hp, :],
                            start=True, stop=True)
                    p2 = pat.tile([P, 2, NK], b16, tag="p2", bufs=2, name="p2")
                    nc.scalar.activation(p2[:], s2[:], AF.Exp, scale=0.125)
                    for j in range(2):
                        for ts4 in range(TT):
                            # one start/stop per 2KB PSUM zero region
                            nc.tensor.matmul(
                                ctxp[j][:, ts4, 0:HD + 1],
                                p2[:, j, ts4 * P:(ts4 + 1) * P],
                                kn_t[hp][:, mt, j, :],
                                start=(mt == 0 and ts4 == 0),
                                stop=(mt == n_mt - 1 and ts4 == TT - 1))
                    if filler is not None:
                        credit += gap_ns
                        while credit > 0:
                            c = next(filler, None)
                            if c is None:
                                filler = None
                                break
                            credit -= c
                rec = small.tile([P, 2, TT, 1], f32, tag="rec")
                for j in range(2):
                    nc.vector.reciprocal(rec[:, j], ctxp[j][:, :, HD:HD + 1])
                ctxh = pat.tile([P, TT, P], b16, tag="ctxh", bufs=2,
                                name="ctxh")
                for j in range(2):
                    for ts4 in range(TT):
                        nc.vector.tensor_scalar(
                            ctxh[:, ts4, j * HD:(j + 1) * HD],
                            ctxp[j][:, ts4, 0:HD],
                            rec[:, j, ts4], None, ALU.mult)
                tp = pa.tile([P, NK], b16, tag="tpb16", bufs=1)
                for ts4 in range(TT):
                    nc.tensor.transpose(tp[:, ts4 * P:(ts4 + 1) * P],
                                        ctxh[:, ts4, :], ident_b[:])
                nc.vector.tensor_copy(ctxT[:, hp, :], tp[:])
            if filler is not None:
                while next(filler, None) is not None:
                    pass

        def ln_tt(rsb, tt, x_out, gkey, post=None):
            """LayerNorm one token tile (keeps the per-tt pipeline flowing)."""
            st = small.tile([P, 2, 6], f32, tag="ln_st1", bufs=4)
            for h in range(2):
                nc.vector.bn_stats(st[:, h, :], rsb[:, tt, h * NK:(h + 1) * NK])
            mv = small.tile([P, 2], f32, tag="ln_mv1", bufs=4)
            nc.vector.bn_aggr(mv[:], st[:])
            tv = small.tile([P, 1], f32, tag="ln_t1", bufs=4)
            nc.vector.tensor_scalar_add(tv[:], mv[:, 1:2], EPS)
            sv = small.tile([P, 1], f32, tag="ln_s1", bufs=4)
            nc.scalar.sqrt(sv[:], tv[:])
            r0 = small.tile([P, 1], f32, tag="ln_r01", bufs=4)
            nc.vector.reciprocal(r0[:], sv[:])
            u = small.tile([P, 1], f32, tag="ln_u1", bufs=4)
            nc.vector.tensor_tensor(out=u[:], in0=tv[:], in1=r0[:], op=ALU.mult)
            nc.vector.tensor_tensor(out=u[:], in0=u[:], in1=r0[:], op=ALU.mult)
            nc.vector.tensor_scalar(u[:], u[:], -0.5, 1.5, ALU.mult, ALU.add)
            r1 = small.tile([P, 1], f32, tag="ln_r11", bufs=4)
            nc.vector.tensor_tensor(out=r1[:], in0=r0[:], in1=u[:], op=ALU.mult)
            if trivial_affine:
                nc.vector.tensor_scalar(
                    x_out[:, tt, :], rsb[:, tt, :], mv[:, 0:1], r1[:, 0:1],
                    ALU.subtract, ALU.mult)
            else:
                g_t = small.tile([P, D], f32, tag="ln_g", bufs=2)
                nc.sync.dma_start(g_t[:], gb_d["g" + gkey][:])
                b_t = small.tile([P, D], f32, tag="ln_b", bufs=2)
                nc.sync.dma_start(b_t[:], gb_d["b" + gkey][:])
                nc.vector.tensor_scalar(
                    rsb[:, tt, :], rsb[:, tt, :], mv[:, 0:1], r1[:, 0:1],
                    ALU.subtract, ALU.mult)
                nc.vector.tensor_tensor(out=rsb[:, tt, :], in0=rsb[:, tt, :],
                                        in1=g_t[:], op=ALU.mult)
                nc.vector.tensor_tensor(out=x_out[:, tt, :], in0=rsb[:, tt, :],
                                        in1=b_t[:], op=ALU.add)
            if post is not None:
                post(tt)

        def o_proj_ln(ctxT, ow_t, resid, x_out, gkey, pp, rsb, extra=None,
                      post_tt=None, xT_out=None, qW=None, QT_out=None):
            """Per-token-tile pipeline: o-proj -> +residual -> LN -> (x^T ->
            Q^T) so the LN scalar chain and the next projection overlap the
            following tile's o-proj matmuls."""
            for tt in range(TT):
                po = pp.tile([P, D], f32, tag="po", bufs=2)
                for cht in range(CHT):
                    for dc in range(2):
                        nc.tensor.matmul(
                            po[:, dc * NK:(dc + 1) * NK],
                            ctxT[:, cht, tt * P:(tt + 1) * P],
                            ow_t[:, cht, dc * NK:(dc + 1) * NK],
                            start=(cht == 0), stop=(cht == CHT - 1))
                nc.vector.tensor_tensor(out=rsb[:, tt, :], in0=po[:],
                                        in1=resid[:, tt, :], op=ALU.add)
                if extra is not None:
                    nc.vector.tensor_tensor(out=rsb[:, tt, :], in0=rsb[:, tt, :],
                                            in1=extra[:], op=ALU.add)
                ln_tt(rsb, tt, x_out, gkey, post=post_tt)
                if xT_out is not None:
                    # x^T for this token tile (all channel tiles)
                    tp = pp.tile([P, CHT, P], rdt, tag="xttp2", bufs=1)
                    for cht in range(CHT):
                        nc.tensor.transpose(
                            tp[:, cht, :],
                            x_out[:, tt, cht * P:(cht + 1) * P], ident_r[:])
                    nc.vector.tensor_copy(xT_out[:, :, tt * P:(tt + 1) * P],
                                          tp[:])
                if QT_out is not None:
                    pq = pp.tile([P, CHT, P], f32, tag="psq2", bufs=1)
                    for cht in range(CHT):
                        for dti in range(DT):
                            nc.tensor.matmul(
                                pq[:, cht, :],
                                qW[:, dti, cht * P:(cht + 1) * P],
                                xT_out[:, dti, tt * P:(tt + 1) * P],
                                start=(cht % 4 == 0 and dti == 0),
                                stop=(cht % 4 == 3 and dti == DT - 1))
                    nc.vector.tensor_copy(QT_out[:, :, tt * P:(tt + 1) * P],
                                          pq[:])

        def transpose_own(x_t, xT, pp):
            """x_t [P, TT, D] (f32r) -> xT [P, CHT, TS] bf16."""
            for cht in range(CHT):
                tp = pp.tile([P, NK], rdt, tag="xttp", bufs=2)
                for tt in range(TT):
                    nc.tensor.transpose(
                        tp[:, tt * P:(tt + 1) * P],
                        x_t[:, tt, cht * P:(cht + 1) * P],
                        ident_r[:])
                nc.vector.tensor_copy(xT[:, cht, :], tp[:])

        # ================= phase S: self-attention =================
        def wload(pool, dram, shape, nm):
            t = pool.tile(shape, b16, tag=nm, name=nm)
            nc.sync.dma_start(t[:], dram[:])
            return t

        # reserve the p2 tag early so `small` sits at the stack bottom
        small.tile([P, 2, NK], b16, tag="p2", bufs=2, name="p2")

        pPerm = tc.alloc_tile_pool(name="sbPerm", bufs=1)     # dies at end
        x2_t = pPerm.tile([P, TT, D], rdt, tag="x2", name="x2")
        # lives through both attention cores (written by the self-core filler)
        pK2T = tc.alloc_tile_pool(name="sbK2T", bufs=1)       # dies post-o-proj2
        K2T_t = pK2T.tile([P, CHT, NC], b16, tag="K2T", name="K2T")
        kn2 = [pK2T.tile([P, NT, 2, HD + 1], b16, tag=f"kn2_{c}",
                         name=f"kn2_{c}") for c in range(CHT)]
        ctxT1 = pK2T.tile([P, CHT, TS], b16, tag="ctxT1", name="ctxT1")
        nc.sync.dma_start(mo_t[:], mo_d[:])
        for c in range(CHT):
            for j in range(2):
                nc.vector.tensor_copy(kn2[c][:, :, j, HD:HD + 1], mo_t[:])

        pEnc = tc.alloc_tile_pool(name="sbEnc", bufs=1)       # dies post-self-core
        encT_t = pEnc.tile([P, DT, NC], b16, tag="encT", name="encT")
        w2W_t = pEnc.tile([P, DT, D], b16, tag="w2W", name="w2W")

        pS1 = tc.alloc_tile_pool(name="sbS1", bufs=1)         # dies post-self-core
        QT_t = pS1.tile([P, CHT, NK], b16, tag="QT", name="QT")
        K1T_t = pS1.tile([P, CHT, L], b16, tag="K1T", name="K1T")
        kn1 = [pS1.tile([P, MT, 2, HD + 1], b16, tag=f"kn1_{c}", name=f"kn1_{c}")
               for c in range(CHT)]
        for c in range(CHT):
            nc.vector.memset(kn1[c][:, :, :, HD:HD + 1], 1.0)

        # DMA order is start-latency-critical: xq + q1W first so the Q
        # projection starts ASAP, then w1W + the xb chunks; encT/w2W ride
        # behind the hot stream (first consumed mid-core by the filler).
        pQ1 = tc.alloc_tile_pool(name="sbQ1", bufs=1)         # dies post-QT-proj
        xq_t = pQ1.tile([P, DT, NK], b16, tag="xq", name="xq")
        nc.sync.dma_start(xq_t[:], xqT_d[:])
        q1W_t = wload(pQ1, q1W_d, [P, DT, D], "q1W")
        with tc.tile_pool(name="ps_proj", bufs=1, space="PSUM") as pp:
            proj_T(q1W_t, xq_t, QT_t, pp, "qt", NK)
        pQ1.release()
        pX = tc.alloc_tile_pool(name="sbX", bufs=1)           # dies post-proj
        w1W_t = wload(pX, w1W_d, [P, DT, D], "w1W")
        XBC = 256  # xbT stream-chunk width (2 m-tiles)
        NXB = L // XBC
        with tc.tile_pool(name="ps_proj1b", bufs=1, space="PSUM") as pp:
            for mc in range(NXB):
                xb_t = pX.tile([P, DT, XBC], b16, tag="xb", bufs=2, name="xb")
                nc.sync.dma_start(xb_t[:], xbT_d[:, :, mc * XBC:(mc + 1) * XBC])
                for cht in range(CHT):
                    ps = pp.tile([P, NK], f32, tag="ps_k1", bufs=3)
                    for dti in range(DT):
                        nc.tensor.matmul(ps[:, 0:XBC],
                                         w1W_t[:, dti, cht * P:(cht + 1) * P],
                                         xb_t[:, dti, :],
                                         start=(dti == 0), stop=(dti == DT - 1))
                    if cht % 2 == 0:
                        nc.vector.tensor_copy(
                            K1T_t[:, cht, mc * XBC:(mc + 1) * XBC], ps[:, 0:XBC])
                    else:
                        nc.scalar.copy(
                            K1T_t[:, cht, mc * XBC:(mc + 1) * XBC], ps[:, 0:XBC])
                    if mc == NXB - 1 and cht == 1:
                        # queue the mid-core loads behind the last hot chunks
                        nc.sync.dma_start(encT_t[:], encT_d[:])
                        nc.sync.dma_start(w2W_t[:], w2W_d[:])
                for cht in range(CHT):
                    if cht % 2 == 0:
                        tp = pp.tile([P, NK], b16, tag="kntp", bufs=2)
                    co = (cht % 2) * XBC
                    for j4 in range(2):
                        mt = mc * 2 + j4
                        nc.tensor.transpose(
                            tp[:, co + j4 * P:co + (j4 + 1) * P],
                            K1T_t[:, cht, mt * P:(mt + 1) * P], ident_b[:])
                    knat_copy(kn1, cht, mc * 2, 2, tp[:, co:co + XBC])
        pX.release()

        # cross-attention K2^T projection + K-natural build, streamed one
        # matmul at a time into the self core's ACT-bound idle gaps
        def k2_filler(pa):
            for cht in range(CHT):
                for ck in range(NKC):
                    w = min(NK, NC - ck * NK)
                    ps = pa.tile([P, NK], f32, tag="ps_k2", bufs=1,
                                 name="ps_k2")
                    for dti in range(DT):
                        nc.tensor.matmul(
                            ps[:, 0:w],
                            w2W_t[:, dti, cht * P:(cht + 1) * P],
                            encT_t[:, dti, ck * NK:ck * NK + w],
                            start=(dti == 0), stop=(dti == DT - 1))
                        yield w * 0.417
                    nc.vector.tensor_copy(
                        K2T_t[:, cht, ck * NK:ck * NK + w], ps[:, 0:w])
                    yield 0.0
                for g0 in range(0, NT, 4):
                    rem = min(4, NT - g0)
                    tp = pa.tile([P, NK], b16, tag="tpb16", bufs=1)
                    for j4 in range(rem):
                        nc.tensor.transpose(
                            tp[:, j4 * P:(j4 + 1) * P],
                            K2T_t[:, cht, (g0 + j4) * P:(g0 + j4 + 1) * P],
                            ident_b[:])
                        yield 54.0
                    knat_copy(kn2, cht, g0, rem, tp[:, 0:rem * P])
                    yield 0.0

        with tc.tile_pool(name="ps_attn", bufs=1, space="PSUM") as pa:
            attn_core(K1T_t, kn1, QT_t, ctxT1, MT, pa, small,
                      filler=k2_filler(pa), gap_ns=370.0)
        pS1.release()
        pEnc.release()

        # ---- o-proj + LN1 + x1^T + Q2^T ----
        pC2 = tc.alloc_tile_pool(name="sbC2", bufs=1)         # dies post-o-proj2
        x1_t = pC2.tile([P, TT, D], rdt, tag="x1", name="x1")
        pO1ph = tc.alloc_tile_pool(name="sbO1ph", bufs=1)     # dies post-Q2T
        o1W_t = wload(pO1ph, o1W_d, [P, DT, D], "o1W")
        xs_t = pO1ph.tile([P, TT, D], f32, tag="xs", name="xs")
        nc.sync.dma_start(xs_t[:], xs_d.rearrange("(lt p) d -> p lt d", p=P))
        q2W_t = wload(pC2, q2W_d, [P, DT, D], "q2W")
        o2W_t = wload(pC2, o2W_d, [P, DT, D], "o2W")
        rsb1 = pO1ph.tile([P, TT, D], f32, tag="rsb1", name="rsb1")
        x1T_t = pO1ph.tile([P, CHT, TS], b16, tag="x1T", name="x1T")
        Q2T_t = pC2.tile([P, CHT, NK], b16, tag="Q2T", name="Q2T")
        with tc.tile_pool(name="ps_o1", bufs=1, space="PSUM") as po:
            o_proj_ln(ctxT1, o1W_t, xs_t, x1_t, "1", po, rsb1,
                      xT_out=x1T_t, qW=q2W_t, QT_out=Q2T_t)
        pO1ph.release()

        # ================= phase C: cross-attention =================
        ctxT2 = pC2.tile([P, CHT, TS], b16, tag="ctxT2", name="ctxT2")
        with tc.tile_pool(name="ps_attn2", bufs=1, space="PSUM") as pa:
            attn_core(K2T_t, kn2, Q2T_t, ctxT2, NT, pa, small)

        pO2ph = tc.alloc_tile_pool(name="sbO2ph", bufs=1)     # dies post-o-proj2
        rsb2 = pO2ph.tile([P, TT, D], f32, tag="rsb2", name="rsb2")
        with tc.tile_pool(name="ps_o2", bufs=1, space="PSUM") as po:
            o_proj_ln(ctxT2, o2W_t, x1_t, x2_t, "2", po, rsb2)
        pO2ph.release()
        pC2.release()
        pK2T.release()

        # ================= phase F: FFN =================
        pFm = tc.alloc_tile_pool(name="sbFm", bufs=1)         # dies at end
        x2T_t = pFm.tile([P, CHT, TS], b16, tag="x2T", name="x2T")
        pFw1 = tc.alloc_tile_pool(name="sbFw1", bufs=1)       # dies post-ff1
        w1c = []
        for wc in range(4):  # 4 tiles -> ff1 starts after the first quarter
            t = pFw1.tile([P, DT, MLP // 4], b16, tag=f"ffW1_{wc}",
                          name=f"ffW1_{wc}")
            nc.sync.dma_start(t[:], ffW1_d[:, :, wc * D:(wc + 1) * D])
            w1c.append(t)
        with tc.tile_pool(name="ps_x2t", bufs=1, space="PSUM") as po:
            transpose_own(x2_t, x2T_t, po)
        hT_t = pFm.tile([P, HTT, TS], b16, tag="hT", name="hT")
        w2_t = pFm.tile([P, HTT, D], b16, tag="ffW2", name="ffW2")
        with tc.tile_pool(name="ps_ffn", bufs=1, space="PSUM") as pf:
            for ht in range(HTT):
                if ht == HTT // 2:
                    # W1 half consumed; start the W2 fetch mid-ff1
                    nc.sync.dma_start(w2_t[:], ffW2_d[:])
                ps = pf.tile([P, NK], f32, tag="ph", bufs=3)
                wt = w1c[ht // (HTT // 4)]
                ho = (ht % (HTT // 4)) * P
                for dti in range(DT):
                    nc.tensor.matmul(ps[:],
                                     wt[:, dti, ho:ho + P],
                                     x2T_t[:, dti, :],
                                     start=(dti == 0), stop=(dti == DT - 1))
                bias = 0.0 if trivial_ffb else ffb1h_t[:, ht:ht + 1]
                nc.scalar.activation(hT_t[:, ht, :], ps[:], AF.Gelu, bias=bias)
        pFw1.release()
        rsb3 = pFm.tile([P, TT, D], b16, tag="rsb3", name="rsb3")
        if not trivial_ffb:
            ffb2c = pFm.tile([P, D], f32, tag="ffb2", name="ffb2")
            nc.sync.dma_start(ffb2c[:], ffb2b_d[:])
        with tc.tile_pool(name="ps_ff2", bufs=1, space="PSUM") as pf:
            for tt in range(TT):
                pff = pf.tile([P, D], f32, tag="pf2", bufs=2)
                for ht in range(HTT):
                    for dc in range(2):
                        nc.tensor.matmul(
                            pff[:, dc * NK:(dc + 1) * NK],
                            hT_t[:, ht, tt * P:(tt + 1) * P],
                            w2_t[:, ht, dc * NK:(dc + 1) * NK],
                            start=(ht == 0), stop=(ht == HTT - 1))
                nc.vector.tensor_tensor(out=rsb3[:, tt, :], in0=pff[:],
                                        in1=x2_t[:, tt, :], op=ALU.add)
                if not trivial_ffb:
                    nc.vector.tensor_tensor(out=rsb3[:, tt, :],
                                            in0=rsb3[:, tt, :],
                                            in1=ffb2c[:], op=ALU.add)
                ln_tt(rsb3, tt, rsb3, "3",
                      post=lambda t: nc.sync.dma_start(
                          out_d[t * P:(t + 1) * P, :], rsb3[:, t, :]))
        pFm.release()
        pPerm.release()
        small.release()
        cpool.release()

    lp.__exit__(None, None, None)
    nc.compile()
    return nc


def _pmajor(w, p=P):
    """[R, C] row-major -> [p, R//p, C] partition-major tiling."""
    r, c = w.shape
    return np.ascontiguousarray(w.reshape(r // p, p, c).swapaxes(0, 1))


def _host_prep(inputs):
    import ml_dtypes

    b16 = ml_dtypes.bfloat16
    x = np.asarray(inputs["x"], np.float32)          # [B, L, D]
    enc = np.asarray(inputs["enc_output"], np.float32)
    mask = np.asarray(inputs["mask"])                # [B, 1, M, 1]

    n = np.arange(D) // HD
    d = np.arange(D) % HD
    perm = d * H + n

    def pw(q, w, o):
        return (np.asarray(q, np.float32)[:, perm],
                np.asarray(w, np.float32)[:, perm],
                np.asarray(o, np.float32)[perm, :])

    q1W, w1W, o1W = pw(inputs["q1W"], inputs["w1W"], inputs["o1W"])
    q2W, w2W, o2W = pw(inputs["q2W"], inputs["w2W"], inputs["o2W"])
    ffW1 = np.asarray(inputs["ffW1"], np.float32)
    ffW2 = np.asarray(inputs["ffW2"], np.float32)
    ffb1 = np.asarray(inputs["ffb1"], np.float32)
    ffb2 = np.asarray(inputs["ffb2"], np.float32)
    g = {k: np.asarray(inputs[k], np.float32)
         for k in ("g1", "b1", "g2", "b2", "g3", "b3")}

    trivial_affine = all(
        np.all(g[f"g{i}"] == 1.0) and np.all(g[f"b{i}"] == 0.0) for i in (1, 2, 3))
    trivial_ffb = bool(np.all(ffb1 == 0.0) and np.all(ffb2 == 0.0))

    # Compact the cross-attention context per batch: masked positions are
    # dropped (their softmax weight is exactly 0 in the reference); both
    # batches pad to a common m-tile count NT.  Padded slots contribute
    # exactly 0 on-device via zeroed K-natural ones-columns.
    kept = [np.where(~mask[b, 0, :, 0])[0] for b in range(B)]
    NT = max(1, max((len(k) + P - 1) // P for k in kept))
    ncx = NT * P
    encCT = []
    mos = []
    for b in range(B):
        encC = np.zeros((ncx, D), np.float32)
        encC[0:len(kept[b])] = enc[b][kept[b]]
        mo = np.zeros(ncx, np.float32)
        mo[0:len(kept[b])] = 1.0
        encCT.append(np.ascontiguousarray(
            _pmajor(np.ascontiguousarray(encC.T))).astype(b16))
        mos.append(np.ascontiguousarray(
            mo.reshape(NT, P).T.reshape(P, NT, 1)).astype(b16))

    xbT = [np.ascontiguousarray(
        _pmajor(np.ascontiguousarray(x[b].T))).astype(b16) for b in range(B)]

    wmaps = {
        "q1W": _pmajor(q1W).astype(b16), "w1W": _pmajor(w1W).astype(b16),
        "o1W": _pmajor(o1W).astype(b16),
        "q2W": _pmajor(q2W).astype(b16), "w2W": _pmajor(w2W).astype(b16),
        "o2W": _pmajor(o2W).astype(b16),
        "ffW1": _pmajor(ffW1).astype(b16), "ffW2": _pmajor(ffW2).astype(b16),
    }
    if not trivial_affine:
        for k in ("g1", "b1", "g2", "b2", "g3", "b3"):
            wmaps[k + "b"] = np.ascontiguousarray(
                np.broadcast_to(g[k], (P, D)).astype(np.float32))

    in_maps = []
    for r in range(NCORES):
        b = r // (NCORES // B)
        lo = (r % (NCORES // B)) * TS
        xq = np.ascontiguousarray(x[b, lo:lo + TS].T)  # [D, TS]
        im = dict(wmaps)
        im["xqT"] = np.ascontiguousarray(_pmajor(xq)).astype(b16)
        im["xbT"] = xbT[b]
        im["xs"] = np.ascontiguousarray(x[b, lo:lo + TS]).astype(np.float32)
        im["encT"] = encCT[b]
        im["mo"] = mos[b]
        if not trivial_ffb:
            im["ffb2b"] = np.ascontiguousarray(
                np.broadcast_to(ffb2, (P, D)).astype(np.float32))
            im["ffb1h"] = np.ascontiguousarray(
                ffb1.reshape(HTT, P).T.astype(np.float32))
        in_maps.append(im)
    return in_maps, trivial_affine, trivial_ffb, NT


def kernel(**inputs) -> np.ndarray:
    in_maps, trivial_affine, trivial_ffb, NT = _host_prep(inputs)
    key = (trivial_affine, trivial_ffb, NT)
    if key not in _PROGRAM_CACHE:
        _PROGRAM_CACHE[key] = _build_program(*key)
    nc = _PROGRAM_CACHE[key]
    res = run_bass_kernel_spmd(nc, in_maps, list(range(NCORES)))
    out = np.empty((T, D), np.float32)
    for r in range(NCORES):
        out[r * TS:(r + 1) * TS, :] = res.results[r]["out"].astype(np.float32)
    return out.reshape(B, L, D)
